# revision 1
# baseline (speedup 1.0000x reference)
"""Trainium2 Bass kernel for the ContinuousGRULayer problem.

Data-parallel over batch: 8 cores, 64 batch rows each. The T=512 time
recurrence runs locally per core with the hidden state kept in transposed
layout [H partitions, B free] so every recurrent matmul is a native
lhsT.T @ rhs with no per-step transposes.

Per step (all elementwise tiles live on partitions 0:64, lane-aligned):
  flow layer l:  ps_g = wtt_g (x) tt  (+accumulate)  W_g @ h   for g in {r,z}
                 sig_g = sigmoid(ps_g + b_g)           (ACT, bias fused)
                 u = tanh(W_u @ (sig_r * h) + wtt_u (x) tt + b_u)   [beta in W_u]
                 h += ((alpha*sig_z) * tanh(tw (x) tt)) * (u - h)
  GRU cell:      x-side matmuls accumulate into PSUM off the critical path;
                 n-gate uses fused scalar_tensor_tensor:
                 rhn = (h_n + b_hn)*r ; s = (i_n + b_in) + rhn ; n = tanh(s)
                 h = n + z*(h - n)

tanh(tw (x) tt) is precomputed on PE+ACT in 8-step chunks (rank-1 matmul
tw (x) tt into PSUM, tanh to SBUF), overlapped with the recurrence.

All weights/biases ride in one packed [64, WB_COLS] tensor (single DMA) to
keep per-instruction semaphore wait counts low.
"""

import numpy as np

import concourse.bass as bass
import concourse.bacc as bacc
import concourse.mybir as mybir
from concourse.tile import TileContext
from concourse.bass_utils import run_bass_kernel_spmd

B, T, D, H, L = 512, 512, 32, 64, 2
NCORES = 8
BL = B // NCORES  # 64 batch rows per core
ALPHA, BETA = 2.0 / 5.0, 4.0 / 5.0
FP = mybir.dt.float32
AF = mybir.ActivationFunctionType
OP = mybir.AluOpType

# packed weight layout: name -> (row_count, col_offset, col_width)
_W64 = ["whr0", "whz0", "whu0", "whr1", "whz1", "whu1", "ggr", "ggz", "ggn"]
_W32 = ["gxr", "gxz", "gxn"]
_W1 = ["wtr0", "wtz0", "wtu0", "tw0", "wtr1", "wtz1", "wtu1", "tw1"]
_WB = ["br0", "bz0", "bu0", "br1", "bz1", "bu1", "gbr", "gbz", "gbhn", "gbin"]


def _wb_layout():
    lay, off = {}, 0
    for n in _W64:
        lay[n] = (64, off, 64)
        off += 64
    for n in _W32:
        lay[n] = (32, off, 64)
        off += 64
    for n in _W1:
        lay[n] = (1, off, 64)
        off += 64
    for n in _WB:
        lay[n] = (64, off, 1)
        off += 1
    return lay, off


_WLAY, WB_COLS = _wb_layout()


def _build(t_steps=T, reps=1):
    assert t_steps % 8 == 0
    nchunks = t_steps // 8
    nc = bacc.Bacc("TRN2", debug=False, enable_asserts=False)

    xp = nc.dram_tensor("xp", [D, t_steps * BL], FP, kind="ExternalInput").ap()
    ttf = nc.dram_tensor("ttf", [nchunks, 8 * BL], FP, kind="ExternalInput").ap()
    wb = nc.dram_tensor("wb", [64, WB_COLS], FP, kind="ExternalInput").ap()
    out = nc.dram_tensor("out", [t_steps, H, BL], FP, kind="ExternalOutput").ap()

    with TileContext(nc) as tc:
        with (
            tc.tile_pool(name="const", bufs=1) as cpool,
            tc.tile_pool(name="ps", bufs=6, space="PSUM") as pspool,
            tc.tile_pool(name="taups", bufs=2, space="PSUM") as taupspool,
            tc.tile_pool(name="sb", bufs=3) as sbpool,
            tc.tile_pool(name="taopool", bufs=4) as taupool,
        ):
            x_sb = cpool.tile([D, t_steps * BL], FP, tag="x", name="x_sb")
            nc.sync.dma_start(out=x_sb[:], in_=xp[:])
            wb_sb = cpool.tile([64, WB_COLS], FP, tag="wb", name="wb_sb")
            nc.sync.dma_start(out=wb_sb[:], in_=wb[:])

            def W(name):
                r, o, w = _WLAY[name]
                return wb_sb[0:r, o:o + w]

            for _rep in range(reps):
              h_cur = sbpool.tile([H, BL], FP, tag="h", bufs=4, name="h0")
              nc.vector.memset(h_cur[:], 0.0)

              tau = [None, None]
              ttchunk = None
              for t in range(t_steps):
                  if t % 8 == 0:
                      c = t // 8
                      # stage this chunk's tt values at partition 0 for matmuls
                      ttchunk = sbpool.tile([1, 8 * BL], FP, tag="ttc", bufs=3,
                                            name="ttc")
                      nc.sync.dma_start(out=ttchunk[:], in_=ttf[c:c + 1, :])
                      for l in range(L):
                          tps = taupspool.tile([H, 8 * BL], FP, tag="taups",
                                               name="taups")
                          nc.tensor.matmul(tps[:], W(f"tw{l}"), ttchunk[:],
                                           start=True, stop=True)
                          tau_t = taupool.tile([H, 8 * BL], FP, tag=f"tau{l}",
                                               name=f"tau{l}")
                          nc.scalar.activation(tau_t[:], tps[:], AF.Tanh)
                          nc.vector.tensor_scalar_mul(tau_t[:], tau_t[:], ALPHA)
                          tau[l] = tau_t
                  ttrow = ttchunk[0:1, (t % 8) * BL:(t % 8 + 1) * BL]
                  toff = (t % 8) * BL

                  # ---- flow layers (the hiddens output is the post-flow state)
                  for l in range(L):
                      ps_r = pspool.tile([H, BL], FP, tag="ps", name="ps_r")
                      nc.tensor.matmul(ps_r[:], W(f"wtr{l}"), ttrow,
                                       start=True, stop=False)
                      nc.tensor.matmul(ps_r[:], W(f"whr{l}"), h_cur[:],
                                       start=False, stop=True)
                      ps_z = pspool.tile([H, BL], FP, tag="ps", name="ps_z")
                      nc.tensor.matmul(ps_z[:], W(f"wtz{l}"), ttrow,
                                       start=True, stop=False)
                      nc.tensor.matmul(ps_z[:], W(f"whz{l}"), h_cur[:],
                                       start=False, stop=True)
                      sr = sbpool.tile([H, BL], FP, tag="sr", name="sr")
                      nc.scalar.activation(sr[:], ps_r[:], AF.Sigmoid,
                                           bias=W(f"br{l}"))
                      sz = sbpool.tile([H, BL], FP, tag="sz", name="sz")
                      nc.scalar.activation(sz[:], ps_z[:], AF.Sigmoid,
                                           bias=W(f"bz{l}"))
                      # g = (alpha*sig_z) * tanh(tw (x) tt): off the critical path
                      g = sbpool.tile([H, BL], FP, tag="g", name="g")
                      nc.gpsimd.tensor_mul(g[:], sz[:], tau[l][:, toff:toff + BL])
                      rh = sbpool.tile([H, BL], FP, tag="rh", name="rh")
                      nc.vector.tensor_mul(rh[:], sr[:], h_cur[:])
                      ps_u = pspool.tile([H, BL], FP, tag="ps", name="ps_u")
                      nc.tensor.matmul(ps_u[:], W(f"wtu{l}"), ttrow,
                                       start=True, stop=False)
                      nc.tensor.matmul(ps_u[:], W(f"whu{l}"), rh[:],
                                       start=False, stop=True)
                      u = sbpool.tile([H, BL], FP, tag="u", name="u")
                      nc.scalar.activation(u[:], ps_u[:], AF.Tanh,
                                           bias=W(f"bu{l}"))
                      dd = sbpool.tile([H, BL], FP, tag="dd", name="dd")
                      nc.vector.tensor_sub(dd[:], u[:], h_cur[:])
                      ee = sbpool.tile([H, BL], FP, tag="ee", name="ee")
                      nc.vector.tensor_mul(ee[:], g[:], dd[:])
                      h_new = sbpool.tile([H, BL], FP, tag="h", bufs=4,
                                          name="hf")
                      nc.vector.tensor_add(h_new[:], h_cur[:], ee[:])
                      h_cur = h_new

                  nc.sync.dma_start(out=out[t], in_=h_cur[:])

                  # ---- GRU cell (next step's carry; not needed after last step)
                  if t < t_steps - 1:
                      xs = x_sb[:, t * BL:(t + 1) * BL]
                      ps_gr = pspool.tile([H, BL], FP, tag="ps", name="ps_gr")
                      nc.tensor.matmul(ps_gr[:], W("gxr"), xs,
                                       start=True, stop=False)
                      nc.tensor.matmul(ps_gr[:], W("ggr"), h_cur[:],
                                       start=False, stop=True)
                      ps_gz = pspool.tile([H, BL], FP, tag="ps", name="ps_gz")
                      nc.tensor.matmul(ps_gz[:], W("gxz"), xs,
                                       start=True, stop=False)
                      nc.tensor.matmul(ps_gz[:], W("ggz"), h_cur[:],
                                       start=False, stop=True)
                      gsr = sbpool.tile([H, BL], FP, tag="sr", name="gsr")
                      nc.scalar.activation(gsr[:], ps_gr[:], AF.Sigmoid,
                                           bias=W("gbr"))
                      gsz = sbpool.tile([H, BL], FP, tag="sz", name="gsz")
                      nc.scalar.activation(gsz[:], ps_gz[:], AF.Sigmoid,
                                           bias=W("gbz"))
                      ps_in = pspool.tile([H, BL], FP, tag="ps", name="ps_in")
                      nc.tensor.matmul(ps_in[:], W("gxn"), xs,
                                       start=True, stop=True)
                      ps_hn = pspool.tile([H, BL], FP, tag="ps", name="ps_hn")
                      nc.tensor.matmul(ps_hn[:], W("ggn"), h_cur[:],
                                       start=True, stop=True)
                      rhn = sbpool.tile([H, BL], FP, tag="rhn", name="rhn")
                      nc.vector.scalar_tensor_tensor(
                          rhn[:], ps_hn[:], W("gbhn"), gsr[:],
                          op0=OP.add, op1=OP.mult)
                      s = sbpool.tile([H, BL], FP, tag="s", name="s")
                      nc.vector.scalar_tensor_tensor(
                          s[:], ps_in[:], W("gbin"), rhn[:],
                          op0=OP.add, op1=OP.add)
                      n_t = sbpool.tile([H, BL], FP, tag="n", name="n")
                      nc.scalar.activation(n_t[:], s[:], AF.Tanh)
                      dn = sbpool.tile([H, BL], FP, tag="dd", name="dn")
                      nc.vector.tensor_sub(dn[:], h_cur[:], n_t[:])
                      en = sbpool.tile([H, BL], FP, tag="ee", name="en")
                      nc.vector.tensor_mul(en[:], gsz[:], dn[:])
                      h_new = sbpool.tile([H, BL], FP, tag="h", bufs=4,
                                          name="hg")
                      nc.vector.tensor_add(h_new[:], n_t[:], en[:])
                      h_cur = h_new
    nc.compile()
    return nc


_NC_CACHE = {}


def _get_nc(t_steps=T, reps=1):
    key = (t_steps, reps)
    if key not in _NC_CACHE:
        _NC_CACHE[key] = _build(t_steps, reps)
    return _NC_CACHE[key]


def _pack_weights(inputs):
    f32 = lambda a: np.ascontiguousarray(np.asarray(a, np.float32))
    W_hr, b_hr = f32(inputs["flow_W_hr"]), f32(inputs["flow_b_hr"])
    W_hz, b_hz = f32(inputs["flow_W_hz"]), f32(inputs["flow_b_hz"])
    W_hh, b_hh = f32(inputs["flow_W_hh"]), f32(inputs["flow_b_hh"])
    tw = f32(inputs["flow_tw"])
    gW_ih, gW_hh = f32(inputs["gru_W_ih"]), f32(inputs["gru_W_hh"])
    gb_ih, gb_hh = f32(inputs["gru_b_ih"]), f32(inputs["gru_b_hh"])
    m = {}
    for l in range(L):
        m[f"whr{l}"] = W_hr[l][:, :H].T
        m[f"whz{l}"] = W_hz[l][:, :H].T
        m[f"wtr{l}"] = W_hr[l][:, H][None]
        m[f"wtz{l}"] = W_hz[l][:, H][None]
        m[f"br{l}"] = b_hr[l][:, None]
        m[f"bz{l}"] = b_hz[l][:, None]
        m[f"whu{l}"] = (BETA * W_hh[l][:, :H]).T
        m[f"wtu{l}"] = W_hh[l][:, H][None]
        m[f"bu{l}"] = b_hh[l][:, None]
        m[f"tw{l}"] = tw[l][None]
    m["ggr"] = gW_hh[0:H].T
    m["ggz"] = gW_hh[H:2 * H].T
    m["ggn"] = gW_hh[2 * H:].T
    m["gxr"] = gW_ih[0:H].T
    m["gxz"] = gW_ih[H:2 * H].T
    m["gxn"] = gW_ih[2 * H:].T
    m["gbr"] = (gb_ih + gb_hh)[0:H][:, None]
    m["gbz"] = (gb_ih + gb_hh)[H:2 * H][:, None]
    m["gbhn"] = gb_hh[2 * H:][:, None]
    m["gbin"] = gb_ih[2 * H:][:, None]
    wbarr = np.zeros((64, WB_COLS), np.float32)
    for name, (r, o, w) in _WLAY.items():
        arr = m[name]
        assert arr.shape == (r, w), (name, arr.shape, (r, w))
        wbarr[0:r, o:o + w] = arr
    return wbarr


def make_in_maps(inputs, t_steps=T):
    x = np.asarray(inputs["x"], np.float32)
    t = np.asarray(inputs["t"], np.float32)
    wbarr = _pack_weights(inputs)
    in_maps = []
    for c in range(NCORES):
        bs, be = c * BL, (c + 1) * BL
        xc = x[bs:be, :t_steps].transpose(2, 1, 0).reshape(D, t_steps * BL)
        ttc = t[bs:be, :t_steps, 0].T.reshape(t_steps // 8, 8 * BL)
        in_maps.append({"xp": np.ascontiguousarray(xc),
                        "ttf": np.ascontiguousarray(ttc), "wb": wbarr})
    return in_maps


def run(inputs, t_steps=T, reps=1, **kw):
    nc = _get_nc(t_steps, reps)
    res = run_bass_kernel_spmd(nc, make_in_maps(inputs, t_steps),
                               core_ids=list(range(NCORES)), **kw)
    outs = [res.results[c]["out"].reshape(t_steps, H, BL).transpose(2, 0, 1)
            for c in range(NCORES)]
    return np.concatenate(outs, 0).astype(np.float32), res


def kernel(**inputs):
    o, _ = run(inputs)
    return o



# revision 6
# speedup vs baseline: 1073.5490x; 1073.5490x over previous
"""Restructured ContinuousGRU kernel: shortest dependency chain per step.

The recurrence is latency-bound: per step, a serial chain of cross-engine
dependent instructions. This version cuts the chain to ~18 links via:

1. Split-matmul junctions: the next stage's matmul consumes the UPDATE
   TERMS of h instead of waiting for h to materialize:
     flow:  h' = h + a + c,  a = alpha*g.u (on-path), c = -alpha*g.h
            (off-path);  W@h' accumulates as W@hc + W@a with hc = h + c.
     GRU:   h' = n + z.h - z.n;  W@h' = W@n + W@zh - W@zn, where zh is
            off-path and only zn (one DVE op after tanh) is on-path.
2. All sigmoid/tanh preacts accumulate in PSUM (rank-1 tt terms + x-side
   terms land first, off the critical path); ACT reads PSUM with bias
   fused, so PE -> ACT is a single link.
3. Off-path work (g, c, hc, zh) runs on Pool; h materialization (for the
   elementwise r-gate products) on DVE right after `a`, one link off-path.
4. tau = tanh(tw*t) precomputed per 8-step chunk on PE+ACT.
5. Output written per chunk ([64, 512] tiles, one DMA per 8 steps).

Data-parallel over batch: 8 cores x 64 rows, state kept transposed
[H partitions, B free].
"""

import numpy as np

import concourse.bass as bass
import concourse.bacc as bacc
import concourse.mybir as mybir
from concourse.tile import TileContext
from concourse.bass_utils import run_bass_kernel_spmd

B, T, D, H, L = 512, 512, 32, 64, 2
NCORES = 8
BL = B // NCORES
ALPHA, BETA = 2.0 / 5.0, 4.0 / 5.0
FP = mybir.dt.float32
AF = mybir.ActivationFunctionType
OP = mybir.AluOpType

_W64 = ["whr0", "whz0", "whu0", "whr1", "whz1", "whu1",
        "nwhr0", "nwhz0", "ggr", "ggz", "ggn"]
_W32 = ["gxr", "gxz", "gxn"]
_W1 = ["wtr0", "wtz0", "wtu0", "tw0", "wtr1", "wtz1", "wtu1", "tw1"]
_WB = ["br0", "bz0", "bu0", "br1", "bz1", "bu1",
       "gbr", "gbz", "gbhn", "gbin"]


def _wb_layout():
    lay, off = {}, 0
    for n in _W64:
        lay[n] = (64, off, 64)
        off += 64
    for n in _W32:
        lay[n] = (32, off, 64)
        off += 64
    for n in _W1:
        lay[n] = (1, off, 64)
        off += 64
    for n in _WB:
        lay[n] = (64, off, 1)
        off += 1
    return lay, off


_WLAY, WB_COLS = _wb_layout()


def _build(t_steps=T, reps=1):
    assert t_steps % 8 == 0
    nchunks = t_steps // 8
    nc = bacc.Bacc("TRN2", debug=False, enable_asserts=False)

    xp = nc.dram_tensor("xp", [D, t_steps * BL], FP, kind="ExternalInput").ap()
    ttf = nc.dram_tensor("ttf", [nchunks, 8 * BL], FP,
                         kind="ExternalInput").ap()
    wb = nc.dram_tensor("wb", [64, WB_COLS], FP, kind="ExternalInput").ap()
    out = nc.dram_tensor("out", [nchunks, H, 8 * BL], FP,
                         kind="ExternalOutput").ap()

    with TileContext(nc) as tc:
        with (
            tc.tile_pool(name="const", bufs=1) as cpool,
            tc.tile_pool(name="ps", bufs=6, space="PSUM") as pspool,
            tc.tile_pool(name="taups", bufs=1, space="PSUM") as taupspool,
            tc.tile_pool(name="sb", bufs=3) as sbpool,
            tc.tile_pool(name="taopool", bufs=4) as taupool,
            tc.tile_pool(name="outp", bufs=3) as outpool,
        ):
            x_sb = cpool.tile([D, t_steps * BL], FP, tag="x", name="x_sb")
            nc.sync.dma_start(out=x_sb[:], in_=xp[:])
            wb_sb = cpool.tile([64, WB_COLS], FP, tag="wb", name="wb_sb")
            nc.sync.dma_start(out=wb_sb[:], in_=wb[:])

            def W(name):
                r, o, w = _WLAY[name]
                return wb_sb[0:r, o:o + w]

            def mm(ps, wname, rhs, start, stop):
                nc.tensor.matmul(ps[:], W(wname), rhs, start=start, stop=stop)

            for _rep in range(reps):
                hg = sbpool.tile([H, BL], FP, tag="hg", bufs=4, name="hg0")
                nc.vector.memset(hg[:], 0.0)

                nchunk = t_steps // 8

                def load_ttchunk(c):
                    tt = sbpool.tile([1, 8 * BL], FP, tag="ttc", bufs=3,
                                     name="ttc")
                    nc.sync.dma_start(out=tt[:], in_=ttf[c:c + 1, :])
                    return tt

                def tau_mm(ttc):
                    # both layers' tau preacts, PSUM-resident
                    tps = [None, None]
                    for l in range(L):
                        tp = taupspool.tile([H, 8 * BL], FP, tag=f"taups{l}",
                                            name="taups")
                        nc.tensor.matmul(tp[:], W(f"tw{l}"), ttc[:],
                                         start=True, stop=True)
                        tps[l] = tp
                    return tps

                def tau_alloc():
                    return [taupool.tile([H, 8 * BL], FP, tag=f"tau{l}",
                                         name=f"tau{l}") for l in range(L)]

                def tau_tanh(tps, taus, lo, hi):
                    for l in range(L):
                        nc.scalar.activation(taus[l][:, lo:hi],
                                             tps[l][:, lo:hi], AF.Tanh)

                # prologue: chunk 0's tau computed up front
                ttchunk = load_ttchunk(0)
                tau_ps = tau_mm(ttchunk)
                tau = tau_alloc()
                tau_tanh(tau_ps, tau, 0, 8 * BL)
                ttchunk_nxt = tau_ps_nxt = tau_nxt = None
                out_chunk = None
                n_t = zq = zh = None
                for t in range(t_steps):
                    toff = (t % 8) * BL
                    ph = t % 8
                    if ph == 0:
                        c = t // 8
                        if c > 0:
                            # tau for this chunk was built during chunk c-1
                            ttchunk, tau = ttchunk_nxt, tau_nxt
                            ttchunk_nxt = tau_ps_nxt = tau_nxt = None
                        if c + 1 < nchunk:
                            ttchunk_nxt = load_ttchunk(c + 1)
                        out_chunk = outpool.tile([H, 8 * BL], FP, tag="oc",
                                                 name="oc")
                    elif ph == 1 and ttchunk_nxt is not None:
                        tau_ps_nxt = tau_mm(ttchunk_nxt)
                        tau_nxt = tau_alloc()
                    elif 2 <= ph <= 5 and tau_nxt is not None:
                        # spread the 2x[64,512] tanh over 4 idle ACT slots
                        q = (ph - 2) * 2 * BL
                        tau_tanh(tau_ps_nxt, tau_nxt, q, q + 2 * BL)
                    ttrow = ttchunk[0:1, toff:toff + BL]

                    # th0 = tau0 * hg: ready long before the sigmoids land
                    th0 = sbpool.tile([H, BL], FP, tag="th0", name="th0")
                    nc.gpsimd.tensor_mul(th0[:], hg[:],
                                         tau[0][:, toff:toff + BL])

                    # ---------- flow layer 0 (junction from prev GRU)
                    ps_r0 = pspool.tile([H, BL], FP, tag="ps", name="ps_r0")
                    ps_z0 = pspool.tile([H, BL], FP, tag="ps", name="ps_z0")
                    first = t == 0
                    mm(ps_r0, "wtr0", ttrow, True, first)
                    mm(ps_z0, "wtz0", ttrow, True, first)
                    if not first:
                        mm(ps_r0, "whr0", zh[:], False, False)
                        mm(ps_z0, "whz0", zh[:], False, False)
                        mm(ps_r0, "nwhr0", zq[:], False, True)
                        mm(ps_z0, "nwhz0", zq[:], False, True)
                    s_r0 = sbpool.tile([H, BL], FP, tag="sr0", name="s_r0")
                    nc.scalar.activation(s_r0[:], ps_r0[:], AF.Sigmoid,
                                         bias=W("br0"))
                    s_z0 = sbpool.tile([H, BL], FP, tag="sz0", name="s_z0")
                    nc.scalar.activation(s_z0[:], ps_z0[:], AF.Sigmoid,
                                         bias=W("bz0"))
                    rh0 = sbpool.tile([H, BL], FP, tag="rh0", name="rh0")
                    nc.vector.tensor_mul(rh0[:], s_r0[:], hg[:])
                    g0 = sbpool.tile([H, BL], FP, tag="g0", name="g0")
                    nc.gpsimd.tensor_mul(g0[:], s_z0[:],
                                         tau[0][:, toff:toff + BL])
                    # hc0 = hg + (s_z0 * -alpha) * th0 = hg - alpha*g0*hg
                    cz0 = sbpool.tile([H, BL], FP, tag="c0", name="cz0")
                    nc.vector.scalar_tensor_tensor(
                        cz0[:], s_z0[:], -ALPHA, th0[:],
                        op0=OP.mult, op1=OP.mult)
                    hc0 = sbpool.tile([H, BL], FP, tag="hc0", name="hc0")
                    nc.vector.tensor_add(hc0[:], hg[:], cz0[:])
                    ps_u0 = pspool.tile([H, BL], FP, tag="ps", name="ps_u0")
                    mm(ps_u0, "wtu0", ttrow, True, False)
                    mm(ps_u0, "whu0", rh0[:], False, True)
                    u0 = sbpool.tile([H, BL], FP, tag="u0", name="u0")
                    nc.scalar.activation(u0[:], ps_u0[:], AF.Tanh,
                                         bias=W("bu0"))
                    a0 = sbpool.tile([H, BL], FP, tag="a0", name="a0")
                    nc.vector.scalar_tensor_tensor(
                        a0[:], u0[:], ALPHA, g0[:], op0=OP.mult, op1=OP.mult)
                    h0 = sbpool.tile([H, BL], FP, tag="h0", name="h0")
                    nc.vector.tensor_add(h0[:], hc0[:], a0[:])

                    # ---------- flow layer 1 (junction via hc0 + a0)
                    ps_r1 = pspool.tile([H, BL], FP, tag="ps", name="ps_r1")
                    ps_z1 = pspool.tile([H, BL], FP, tag="ps", name="ps_z1")
                    mm(ps_r1, "wtr1", ttrow, True, False)
                    mm(ps_z1, "wtz1", ttrow, True, False)
                    mm(ps_r1, "whr1", hc0[:], False, False)
                    mm(ps_z1, "whz1", hc0[:], False, False)
                    mm(ps_r1, "whr1", a0[:], False, True)
                    mm(ps_z1, "whz1", a0[:], False, True)
                    s_r1 = sbpool.tile([H, BL], FP, tag="sr1", name="s_r1")
                    nc.scalar.activation(s_r1[:], ps_r1[:], AF.Sigmoid,
                                         bias=W("br1"))
                    s_z1 = sbpool.tile([H, BL], FP, tag="sz1", name="s_z1")
                    nc.scalar.activation(s_z1[:], ps_z1[:], AF.Sigmoid,
                                         bias=W("bz1"))
                    th1 = sbpool.tile([H, BL], FP, tag="th1", name="th1")
                    nc.gpsimd.tensor_mul(th1[:], h0[:],
                                         tau[1][:, toff:toff + BL])
                    rh1 = sbpool.tile([H, BL], FP, tag="rh1", name="rh1")
                    nc.vector.tensor_mul(rh1[:], s_r1[:], h0[:])
                    g1 = sbpool.tile([H, BL], FP, tag="g1", name="g1")
                    nc.gpsimd.tensor_mul(g1[:], s_z1[:],
                                         tau[1][:, toff:toff + BL])
                    cz1 = sbpool.tile([H, BL], FP, tag="c1", name="cz1")
                    nc.vector.scalar_tensor_tensor(
                        cz1[:], s_z1[:], -ALPHA, th1[:],
                        op0=OP.mult, op1=OP.mult)
                    hc1 = sbpool.tile([H, BL], FP, tag="hc1", name="hc1")
                    nc.vector.tensor_add(hc1[:], h0[:], cz1[:])
                    ps_u1 = pspool.tile([H, BL], FP, tag="ps", name="ps_u1")
                    mm(ps_u1, "wtu1", ttrow, True, False)
                    mm(ps_u1, "whu1", rh1[:], False, True)
                    u1 = sbpool.tile([H, BL], FP, tag="u1", name="u1")
                    nc.scalar.activation(u1[:], ps_u1[:], AF.Tanh,
                                         bias=W("bu1"))
                    a1 = sbpool.tile([H, BL], FP, tag="a1", name="a1")
                    nc.vector.scalar_tensor_tensor(
                        a1[:], u1[:], ALPHA, g1[:], op0=OP.mult, op1=OP.mult)
                    h1 = out_chunk[:, toff:toff + BL]
                    nc.vector.tensor_add(h1, hc1[:], a1[:])

                    if t % 8 == 7 or t == t_steps - 1:
                        nc.sync.dma_start(out=out[t // 8], in_=out_chunk[:])

                    # ---------- GRU cell (junction via hc1 + a1)
                    if t < t_steps - 1:
                        xs = x_sb[:, t * BL:(t + 1) * BL]
                        ps_gr = pspool.tile([H, BL], FP, tag="ps",
                                            name="ps_gr")
                        ps_gz = pspool.tile([H, BL], FP, tag="ps",
                                            name="ps_gz")
                        ps_hn = pspool.tile([H, BL], FP, tag="ps",
                                            name="ps_hn")
                        ps_in = pspool.tile([H, BL], FP, tag="ps",
                                            name="ps_in")
                        mm(ps_gr, "gxr", xs, True, False)
                        mm(ps_gz, "gxz", xs, True, False)
                        mm(ps_in, "gxn", xs, True, True)
                        mm(ps_gr, "ggr", hc1[:], False, False)
                        mm(ps_gz, "ggz", hc1[:], False, False)
                        mm(ps_hn, "ggn", hc1[:], True, False)
                        mm(ps_gr, "ggr", a1[:], False, True)
                        mm(ps_gz, "ggz", a1[:], False, True)
                        mm(ps_hn, "ggn", a1[:], False, True)
                        s_gr = sbpool.tile([H, BL], FP, tag="sgr",
                                           name="s_gr")
                        nc.scalar.activation(s_gr[:], ps_gr[:], AF.Sigmoid,
                                             bias=W("gbr"))
                        s_gz = sbpool.tile([H, BL], FP, tag="sgz",
                                           name="s_gz")
                        nc.scalar.activation(s_gz[:], ps_gz[:], AF.Sigmoid,
                                             bias=W("gbz"))
                        rhn = sbpool.tile([H, BL], FP, tag="rhn", name="rhn")
                        nc.vector.scalar_tensor_tensor(
                            rhn[:], ps_hn[:], W("gbhn"), s_gr[:],
                            op0=OP.add, op1=OP.mult)
                        s_n = sbpool.tile([H, BL], FP, tag="s", name="s_n")
                        nc.vector.scalar_tensor_tensor(
                            s_n[:], ps_in[:], W("gbin"), rhn[:],
                            op0=OP.add, op1=OP.add)
                        n_t = sbpool.tile([H, BL], FP, tag="n", name="n_t")
                        nc.scalar.activation(n_t[:], s_n[:], AF.Tanh)
                        # zq = (z-1).n ; hg = zh - zq = (1-z).n + z.h
                        zq = sbpool.tile([H, BL], FP, tag="zq", name="zq")
                        nc.vector.scalar_tensor_tensor(
                            zq[:], s_gz[:], 1.0, n_t[:],
                            op0=OP.subtract, op1=OP.mult)
                        zh = sbpool.tile([H, BL], FP, tag="zh", name="zh")
                        nc.gpsimd.tensor_mul(zh[:], s_gz[:], h1)
                        hg = sbpool.tile([H, BL], FP, tag="hg", bufs=4,
                                         name="hg")
                        nc.vector.tensor_sub(hg[:], zh[:], zq[:])
    nc.compile()
    return nc


_NC_CACHE = {}


def _get_nc(t_steps=T, reps=1):
    key = (t_steps, reps)
    if key not in _NC_CACHE:
        _NC_CACHE[key] = _build(t_steps, reps)
    return _NC_CACHE[key]


def _pack_weights(inputs):
    f32 = lambda a: np.ascontiguousarray(np.asarray(a, np.float32))
    W_hr, b_hr = f32(inputs["flow_W_hr"]), f32(inputs["flow_b_hr"])
    W_hz, b_hz = f32(inputs["flow_W_hz"]), f32(inputs["flow_b_hz"])
    W_hh, b_hh = f32(inputs["flow_W_hh"]), f32(inputs["flow_b_hh"])
    tw = f32(inputs["flow_tw"])
    gW_ih, gW_hh = f32(inputs["gru_W_ih"]), f32(inputs["gru_W_hh"])
    gb_ih, gb_hh = f32(inputs["gru_b_ih"]), f32(inputs["gru_b_hh"])
    m = {}
    for l in range(L):
        m[f"whr{l}"] = W_hr[l][:, :H].T
        m[f"whz{l}"] = W_hz[l][:, :H].T
        m[f"wtr{l}"] = W_hr[l][:, H][None]
        m[f"wtz{l}"] = W_hz[l][:, H][None]
        m[f"br{l}"] = b_hr[l][:, None]
        m[f"bz{l}"] = b_hz[l][:, None]
        m[f"whu{l}"] = (BETA * W_hh[l][:, :H]).T
        m[f"wtu{l}"] = W_hh[l][:, H][None]
        m[f"bu{l}"] = b_hh[l][:, None]
        m[f"tw{l}"] = tw[l][None]
    m["nwhr0"] = -m["whr0"]
    m["nwhz0"] = -m["whz0"]
    m["ggr"] = gW_hh[0:H].T
    m["ggz"] = gW_hh[H:2 * H].T
    m["ggn"] = gW_hh[2 * H:].T
    m["gxr"] = gW_ih[0:H].T
    m["gxz"] = gW_ih[H:2 * H].T
    m["gxn"] = gW_ih[2 * H:].T
    m["gbr"] = (gb_ih + gb_hh)[0:H][:, None]
    m["gbz"] = (gb_ih + gb_hh)[H:2 * H][:, None]
    m["gbhn"] = gb_hh[2 * H:][:, None]
    m["gbin"] = gb_ih[2 * H:][:, None]
    wbarr = np.zeros((64, WB_COLS), np.float32)
    for name, (r, o, w) in _WLAY.items():
        arr = m[name]
        assert arr.shape == (r, w), (name, arr.shape, (r, w))
        wbarr[0:r, o:o + w] = arr
    return wbarr


def make_in_maps(inputs, t_steps=T):
    x = np.asarray(inputs["x"], np.float32)
    t = np.asarray(inputs["t"], np.float32)
    wbarr = _pack_weights(inputs)
    in_maps = []
    for c in range(NCORES):
        bs, be = c * BL, (c + 1) * BL
        xc = x[bs:be, :t_steps].transpose(2, 1, 0).reshape(D, t_steps * BL)
        ttc = t[bs:be, :t_steps, 0].T.reshape(t_steps // 8, 8 * BL)
        in_maps.append({"xp": np.ascontiguousarray(xc),
                        "ttf": np.ascontiguousarray(ttc), "wb": wbarr})
    return in_maps


def _unshard(res, t_steps):
    nchunks = t_steps // 8
    outs = []
    for c in range(NCORES):
        o = res.results[c]["out"].reshape(nchunks, H, 8, BL)
        o = o.transpose(0, 2, 1, 3).reshape(t_steps, H, BL)
        outs.append(o.transpose(2, 0, 1))
    return np.concatenate(outs, 0).astype(np.float32)


# ---------------------------------------------------------------------------
# Execution: jit the bass_exec body once per (t_steps, reps) and reuse it.
# Output zero-buffers are staged on device once (run_bass_kernel_spmd would
# re-upload them every call); only x/t/weights transfer per call.

import jax
from jax.sharding import Mesh, NamedSharding, PartitionSpec
from jax.experimental.shard_map import shard_map

import concourse.mybir as _mybir
from concourse import bass2jax as _b2j


class _Runner:
    def __init__(self, nc, n_cores):
        _b2j.install_neuronx_cc_hook()
        self.nc = nc
        self.n_cores = n_cores
        pname = nc.partition_id_tensor.name if nc.partition_id_tensor else None
        in_names, out_names, out_avals = [], [], []
        for alloc in nc.m.functions[0].allocations:
            if not isinstance(alloc, _mybir.MemoryLocationSet):
                continue
            name = alloc.memorylocations[0].name
            if alloc.kind == "ExternalInput":
                if name != pname:
                    in_names.append(name)
            elif alloc.kind == "ExternalOutput":
                out_names.append(name)
                out_avals.append(jax.core.ShapedArray(
                    tuple(alloc.tensor_shape), _mybir.dt.np(alloc.dtype)))
        self.in_names, self.out_names, self.out_avals = \
            in_names, out_names, out_avals
        all_in = list(in_names) + list(out_names)
        if pname is not None:
            all_in.append(pname)

        def _body(*args):
            operands = list(args)
            if pname is not None:
                operands.append(_b2j.partition_id_tensor())
            return tuple(_b2j._bass_exec_p.bind(
                *operands,
                out_avals=tuple(out_avals),
                in_names=tuple(all_in),
                out_names=tuple(out_names),
                lowering_input_output_aliases=(),
                sim_require_finite=True,
                sim_require_nnan=True,
                nc=nc,
            ))

        devices = jax.devices()[:n_cores]
        mesh = Mesh(np.asarray(devices), ("core",))
        nio = len(in_names) + len(out_names)
        self.jitted = jax.jit(
            shard_map(_body, mesh=mesh,
                      in_specs=(PartitionSpec("core"),) * nio,
                      out_specs=(PartitionSpec("core"),) * len(out_names),
                      check_rep=False),
            keep_unused=True)
        self.sharding = NamedSharding(mesh, PartitionSpec("core"))
        self.zeros = [
            jax.device_put(
                np.zeros((n_cores * a.shape[0], *a.shape[1:]), a.dtype),
                self.sharding)
            for a in out_avals]

    def stage(self, in_maps):
        concat = [np.concatenate([np.asarray(m[n]) for m in in_maps], 0)
                  for n in self.in_names]
        return [jax.device_put(a, self.sharding) for a in concat]

    def execute(self, staged):
        outs = self.jitted(*staged, *self.zeros)
        jax.block_until_ready(outs)
        return outs

    def unpack(self, outs):
        return [
            {n: np.asarray(outs[i]).reshape(
                self.n_cores, *self.out_avals[i].shape)[c]
             for i, n in enumerate(self.out_names)}
            for c in range(self.n_cores)]


_RUNNERS = {}


def _get_runner(t_steps=T, reps=1):
    key = (t_steps, reps)
    if key not in _RUNNERS:
        _RUNNERS[key] = _Runner(_get_nc(t_steps, reps), NCORES)
    return _RUNNERS[key]


def run(inputs, t_steps=T, reps=1, **kw):
    r = _get_runner(t_steps, reps)
    outs = r.unpack(r.execute(r.stage(make_in_maps(inputs, t_steps))))

    class _Res:
        pass

    res = _Res()
    res.results = outs
    res.exec_time_ns = None
    return _unshard(res, t_steps), res


def bench_exec(inputs, t_steps=T, reps=1, iters=6, warmup=2):
    """Time device execution only (inputs staged once). Returns seconds."""
    import time as _time
    r = _get_runner(t_steps, reps)
    staged = r.stage(make_in_maps(inputs, t_steps))
    for _ in range(warmup):
        outs = r.execute(staged)
    times = []
    for _ in range(iters):
        t0 = _time.perf_counter()
        outs = r.execute(staged)
        times.append(_time.perf_counter() - t0)

    class _Res:
        pass

    res = _Res()
    res.results = r.unpack(outs)
    return times, _unshard(res, t_steps)


def kernel(**inputs):
    o, _ = run(inputs)
    return o
